# revision 31
# baseline (speedup 1.0000x reference)
"""TRN2 Bass kernel for GQA attention (nn_Attention_19533511262498).

Tensor-parallel over heads across 8 NeuronCores: core c owns q-heads
[4c, 4c+4) and kv-head c (wq/wk/wv sharded on the head dim, wo on its
input dim). Each core computes a partial [S, DIM] output (bf16); the
host sums the 8 partials in f32. All matmuls run bf16 x bf16;
end-to-end max-rel error vs the f32 reference is ~5e-3 (gate 2e-2).

Phase 1 (projections + RoPE): s-blocks of 512, x streamed as k-quarter
tiles; all 6 output blocks (4 q heads, k, v) accumulate concurrently in
6 PSUM banks. All weights live in one half-major packed tensor
(chunks 0-15 / 16-31) so each 0.5MB load lands just before its first
PE deadline across the two hwdge queues; cos/sin are preloaded in full
before wo so RoPE never waits on DMA. RoPE runs entirely on the vector
engine: two half-tile muls with partition-shifted outputs read the
PSUM directly (no SBUF-SBUF swap DMAs). V is transposed by SBUF->SBUF
XBAR DMA transposes as each s-block finishes (no PE transposes).
q-block 0's attention scores+exps+bf16 col-sums are injected between
s-block boundaries of sb1/sb2 (they only need sb0 data and the spare
scps PSUM banks, and must not interleave inside an open accumulation
quarter - that corrupts the qkv PSUM groups on hardware).

Phase 2 (attention + output projection): causal triangle at 128-column
granularity: diagonal kv-chunks compute only the valid column suffix,
with the [128,128] triangle mask accumulated by a PE-local idb x hm
matmul (cheaper in practice than any cross-engine mask: the
score->exp chain stays on-engine; DVE and gpsimd variants both lost
more to chain latency than they saved in PE cycles). Softmax sums
accumulate directly in bf16 (2x DVE rate, and the column-sum matmul
needs bf16 moving anyway); normalizer = ones-column reduce matmul ->
broadcast matmul -> vector reciprocal+mul, with the l copy on the
scalar engine and the held-back half of the projection items bridging
each cross-engine wait. q-block 0 opens phase 2 with dependency-free
pv matmuls over the precomputed exps; its bc/recip/mul chains
interleave between them. The previous q-block's output projection
matmuls are interleaved INTO the score/pv chunk stream so the PE never
waits on the scalar engine's exp chain. The tail (last q-block's
projection, which needs all four heads' outT) runs ob-major across the
4 row-slots with the first three groups' h0-h2 matmuls hoisted ahead
of the last outT's norm chain, and its stores fine-sliced across both
queues. The score-PSUM pool is allocated before the phase-1 QKV pool
(banks 0-1, held across phases) so phase 2 has no WAR on phase-1
banks.

Measured ~389us HW exec (from a 407us baseline); PE active ~360us vs
~345us of pure matmul cycles, remaining idle dominated by the fixed
~7us boot, ~7us teardown, and the cold-DMA startup ramp (finer
startup DMA slicing consistently regressed: more in-flight partial
writes per tile add per-instruction semaphore overhead downstream).
"""

import ml_dtypes
import numpy as np

import concourse.bacc as bacc
import concourse.tile as tile
from concourse import mybir
from concourse.bass import ts, ds
from concourse.bass_utils import run_bass_kernel_spmd

F32 = mybir.dt.float32
BF16 = mybir.dt.bfloat16
NPBF16 = ml_dtypes.bfloat16

# problem geometry (hardcoded per contest rules)
S = 2048
DIM = 4096
HD = 128
N_HEADS = 32
N_KV = 8
NCORES = 8
HPC = N_HEADS // NCORES       # 4 q heads per core
FEAT = HPC * HD               # 512 per-core attention feature width

SBW = 512                     # phase-1 s-block width
NSB = S // SBW                # 4
KCH = DIM // 128              # 32 contraction chunks
KQ = 8                        # k-chunks per x quarter tile
NXQ = KCH // KQ               # 4
QBW = 512                     # attention q-block width
NQB = S // QBW                # 4
NSC = S // 128                # 16 kv chunks
OBW = 512                     # output-dim block width
NOB = DIM // OBW              # 8
OBS = ["k", 0, "v", 1, 2, 3]  # per-sb output-block order (k first)

_CACHE = {}


def _build():
    nc = bacc.Bacc("TRN2", target_bir_lowering=False, debug=False,
                   num_devices=NCORES)

    xT = nc.dram_tensor("xT", [NSB, NXQ, 128, KQ, SBW], BF16,
                        kind="ExternalInput").ap()
    wAllT = nc.dram_tensor("wAllT", [2, 128, 6, 16, HD], BF16,
                           kind="ExternalInput").ap()
    woT = nc.dram_tensor("woT", [HPC, 128, DIM], BF16, kind="ExternalInput").ap()
    cos2 = nc.dram_tensor("cos2", [128, S], F32, kind="ExternalInput").ap()
    sin2 = nc.dram_tensor("sin2", [128, S], F32, kind="ExternalInput").ap()
    sgn = nc.dram_tensor("sgn", [128, 1], F32, kind="ExternalInput").ap()
    onesc = nc.dram_tensor("onesc", [128, 1], BF16, kind="ExternalInput").ap()
    onesr = nc.dram_tensor("onesr", [1, 128], BF16, kind="ExternalInput").ap()
    hm128 = nc.dram_tensor("hm128", [128, 128], BF16, kind="ExternalInput").ap()
    identb = nc.dram_tensor("identb", [128, 128], BF16,
                            kind="ExternalInput").ap()
    out_d = nc.dram_tensor("out", [NQB * HPC, NOB, 128, OBW], BF16,
                           kind="ExternalOutput").ap()

    with tile.TileContext(nc) as tc:
        with (
            tc.tile_pool(name="res", bufs=1) as res,
            tc.tile_pool(name="wo", bufs=1) as wop,
            # scps first so it lands in PSUM banks 0-1 and carries across
            # both phases (phase-2/injected scores never WAR phase-1 banks)
            tc.tile_pool(name="scps", bufs=2, space="PSUM") as scps,
        ):
            kt_sb = [res.tile([128, SBW], BF16, tag=f"kt{i}", name=f"kt{i}")
                     for i in range(NSB)]
            vt_sb = [res.tile([128, SBW], BF16, tag=f"vt{i}", name=f"vt{i}")
                     for i in range(NSB)]
            q_sb = [res.tile([128, HPC, SBW], BF16, tag=f"q{i}", name=f"q{i}")
                    for i in range(NSB)]
            v_t = res.tile([128, NSC, HD], BF16, tag="v")
            sgn_t = res.tile([128, 1], F32, tag="sgn")
            onesc_t = res.tile([128, 1], BF16, tag="onesc")
            onesr_t = res.tile([1, 128], BF16, tag="onesr")
            hm_t = res.tile([128, 128], BF16, tag="hm")
            idb_t = res.tile([128, 128], BF16, tag="idb")
            cos_t = res.tile([128, S], F32, tag="cos")
            sin_t = res.tile([128, S], F32, tag="sin")
            # qb0 precompute: exps, f32 col-sums, bf16 col-sums
            e0 = [[res.tile([128, QBW], BF16, tag=f"e0_{h}_{sc}",
                            name=f"e0_{h}_{sc}") for sc in range(4)]
                  for h in range(HPC)]
            sum0 = [res.tile([128, QBW], BF16, tag=f"sum0_{h}",
                             name=f"sum0_{h}") for h in range(HPC)]

            # ---------------- Phase 1: QKV projections + RoPE ----------------
            with (
                tc.tile_pool(name="wkv", bufs=1) as wkvp,
                tc.tile_pool(name="xt", bufs=5) as xtp,
                tc.tile_pool(name="rope", bufs=3) as ropep,
                tc.tile_pool(name="qkvps", bufs=6, space="PSUM") as qkvps,
            ):
                wall_t = wkvp.tile([128, 2, 6, 16, HD], BF16, tag="wall")
                xq_tiles = {}

                def load_xq(sb, q, fine=False):
                    t = xtp.tile([128, KQ, SBW], BF16, tag="xt",
                                 name=f"x{sb}_{q}")
                    if fine:
                        for i in range(4):
                            eng = nc.scalar if i % 2 == 0 else nc.sync
                            eng.dma_start(out=t[:, 2 * i:2 * i + 2],
                                          in_=xT[sb, q, :, 2 * i:2 * i + 2])
                    else:
                        nc.sync.dma_start(out=t[:, 0:4], in_=xT[sb, q, :, 0:4])
                        nc.sync.dma_start(out=t[:, 4:8], in_=xT[sb, q, :, 4:8])
                    xq_tiles[(sb, q)] = t

                # half-major weight loads (A = chunks 0-15, B = 16-31),
                # ordered so every ob's A-half beats its first-quarter PE
                # deadline; obi order matches OBS consumption order
                def wload(h, obi, eng, part=None):
                    if part is None:
                        eng.dma_start(out=wall_t[:, h, obi],
                                      in_=wAllT[h, :, obi])
                    else:
                        j0, j1 = part
                        eng.dma_start(out=wall_t[:, h, obi, j0:j1],
                                      in_=wAllT[h, :, obi, j0:j1])

                wload(0, 0, nc.sync)          # A-k
                load_xq(0, 0, fine=True)      # x00 split scalar/sync
                wload(0, 1, nc.scalar)        # A-q0
                wload(0, 3, nc.sync)          # A-q1
                wload(0, 2, nc.scalar)        # A-v
                wload(0, 5, nc.sync)          # A-q3
                wload(0, 4, nc.scalar)        # A-q2
                load_xq(0, 1)                 # x01 on sync
                nc.scalar.dma_start(out=sgn_t, in_=sgn)
                nc.scalar.dma_start(out=onesc_t, in_=onesc)
                nc.scalar.dma_start(out=onesr_t, in_=onesr)
                nc.scalar.dma_start(out=hm_t, in_=hm128)
                nc.scalar.dma_start(out=idb_t, in_=identb)
                load_xq(0, 2)
                wload(1, 0, nc.sync)          # B-k
                wload(1, 1, nc.scalar)        # B-q0
                wload(1, 3, nc.sync)          # B-q1
                wload(1, 2, nc.scalar)        # B-v
                wload(1, 5, nc.sync)          # B-q3
                wload(1, 4, nc.scalar)        # B-q2
                load_xq(0, 3)
                # full trig preload (before wo) so RoPE never waits on DMA
                nc.scalar.dma_start(out=cos_t[:, 0:1024], in_=cos2[:, 0:1024])
                nc.scalar.dma_start(out=cos_t[:, 1024:S], in_=cos2[:, 1024:S])
                nc.scalar.dma_start(out=sin_t[:, 0:1024], in_=sin2[:, 0:1024])
                nc.scalar.dma_start(out=sin_t[:, 1024:S], in_=sin2[:, 1024:S])
                # wo prefetch (4MB bf16) behind the trig
                wo_hs = []
                for h in range(HPC):
                    wo_h = wop.tile([128, DIM], BF16, tag=f"wo{h}")
                    nc.scalar.dma_start(out=wo_h, in_=woT[h])
                    wo_hs.append(wo_h)

                seq = [(sb, q) for sb in range(NSB) for q in range(NXQ)]
                issued = 4  # all of sb0's quarters already issued

                OBI = {"k": 0, 0: 1, "v": 2, 1: 3, 2: 4, 3: 5}

                def wsrc(ob, kc):
                    return wall_t[:, kc // 16, OBI[ob], kc % 16, :]

                # qb0 score/exp/sum injections: emitted between phase-1
                # quarters once sb0's k/q are ready
                def qb0_chunk(h, sc):
                    lo = 128 * sc
                    cols = ds(lo, QBW - lo)
                    s_ps = scps.tile([128, QBW], F32, tag="sc",
                                     name=f"sc0_{h}_{sc}")
                    nc.tensor.matmul(s_ps[:, cols],
                                     kt_sb[0][:, ts(sc, 128)],
                                     q_sb[0][:, h, cols],
                                     start=True, stop=False)
                    nc.tensor.matmul(s_ps[:, ds(lo, 128)], idb_t, hm_t,
                                     start=False, stop=True)
                    nc.scalar.activation(e0[h][sc][:, cols], s_ps[:, cols],
                                         mybir.ActivationFunctionType.Exp)
                    if sc == 0:
                        nc.vector.tensor_copy(sum0[h], e0[h][0])
                    else:
                        nc.vector.tensor_add(sum0[h][:, cols],
                                             sum0[h][:, cols],
                                             e0[h][sc][:, cols])

                inj = []
                for h in range(HPC):
                    for sc in range(4):
                        inj.append(lambda h=h, sc=sc: qb0_chunk(h, sc))

                inj_per_sb = {1: 6, 2: 6}  # rest at phase-1 end
                inj_it = 0

                for sb in range(NSB):
                    c_sl = cos_t[:, ts(sb, SBW)]
                    s_sl = sin_t[:, ts(sb, SBW)]
                    ps = {ob: qkvps.tile([128, SBW], F32, tag="ps",
                                         name=f"ps{sb}_{ob}")
                          for ob in OBS}
                    for q in range(NXQ):
                        # keep 3 quarters of prefetch in flight
                        while issued < len(seq) and issued <= sb * NXQ + q + 3:
                            load_xq(*seq[issued])
                            issued += 1
                        xt = xq_tiles.pop((sb, q))
                        for ob in OBS:
                            for k in range(KQ):
                                kc = q * KQ + k
                                nc.tensor.matmul(ps[ob], wsrc(ob, kc),
                                                 xt[:, k, :],
                                                 start=(kc == 0),
                                                 stop=(kc == KCH - 1))
                    # RoPE / copies in OBS order
                    # dst = m1 + sgn*m2s, m1 = ps*cos,
                    # m2s = half-swapped ps*sin (shifted-output DVE muls)
                    for ob in OBS:
                        if ob == "v":
                            nc.vector.tensor_copy(vt_sb[sb], ps[ob])
                            # XBAR transpose v into [s, hd] chunks
                            for j in range(4):
                                eng = nc.scalar if j % 2 == 0 else nc.sync
                                eng.dma_start(
                                    out=v_t[:, 4 * sb + j, :],
                                    in_=vt_sb[sb][:, ts(j, 128)],
                                    transpose=True)
                            continue
                        m1 = ropep.tile([128, SBW], F32, tag="m1", name="m1")
                        m2s = ropep.tile([128, SBW], F32, tag="m2s",
                                         name="m2s")
                        nc.vector.tensor_mul(m1, ps[ob], c_sl)
                        nc.vector.tensor_tensor(
                            m2s[0:64], ps[ob][64:128], s_sl[64:128],
                            op=mybir.AluOpType.mult)
                        nc.vector.tensor_tensor(
                            m2s[64:128], ps[ob][0:64], s_sl[0:64],
                            op=mybir.AluOpType.mult)
                        dst = (kt_sb[sb] if ob == "k"
                               else q_sb[sb][:, ob, :])
                        nc.vector.scalar_tensor_tensor(
                            dst, m2s, sgn_t, m1,
                            op0=mybir.AluOpType.mult, op1=mybir.AluOpType.add)
                    for _ in range(inj_per_sb.get(sb, 0)):
                        if inj_it < len(inj):
                            inj[inj_it]()
                            inj_it += 1
                while inj_it < len(inj):
                    inj[inj_it]()
                    inj_it += 1

            # ---------------- Phase 2: attention + output projection --------
            with (
                tc.tile_pool(name="exp", bufs=6) as expp,
                tc.tile_pool(name="pair", bufs=3) as pairp,
                tc.tile_pool(name="sum", bufs=3) as sump,
                tc.tile_pool(name="outT", bufs=6) as outTp,
                tc.tile_pool(name="rc", bufs=2) as rcp,
                tc.tile_pool(name="lsb", bufs=2) as lp,
                tc.tile_pool(name="ost", bufs=6) as ostp,
                tc.tile_pool(name="pvps", bufs=2, space="PSUM") as pvps,
                tc.tile_pool(name="normps", bufs=1, space="PSUM") as normps,
                tc.tile_pool(name="prps", bufs=3, space="PSUM") as prps,
            ):
                outT_tiles = {}

                def proj_items(pqb, qs, tail=False):
                    """Yield emit-fns for output-projection slot (pqb, qs)."""
                    slot = pqb * HPC + qs
                    for ob in range(NOB):
                        p_ps = prps.tile([128, OBW], F32, tag="pr",
                                         name=f"pr{pqb}_{qs}_{ob}")
                        for h2 in range(HPC):
                            yield lambda ob=ob, h2=h2, p_ps=p_ps: \
                                nc.tensor.matmul(
                                    p_ps,
                                    outT_tiles[(pqb, h2)][:, ts(qs, 128)],
                                    wo_hs[h2][:, ts(ob, OBW)],
                                    start=(h2 == 0),
                                    stop=(h2 == HPC - 1))

                        # PSUM->SBUF copy (alternate vector/scalar), then a
                        # contiguous 128KB tile store (alternate sync/scalar)
                        def fin(ob=ob, p_ps=p_ps, slot=slot):
                            stg = ostp.tile([128, OBW], BF16, tag="ost",
                                            name="ost")
                            if tail:
                                # vector does all tail copies (scalar queue
                                # must stay free to issue the final stores)
                                nc.vector.tensor_copy(stg, p_ps)
                                if ob >= NOB - 2:
                                    # last groups drain through both queues
                                    nc.sync.dma_start(
                                        out=out_d[slot, ob][:, 0:256],
                                        in_=stg[:, 0:256])
                                    nc.scalar.dma_start(
                                        out=out_d[slot, ob][:, 256:512],
                                        in_=stg[:, 256:512])
                                elif ob % 2 == 0:
                                    nc.sync.dma_start(out=out_d[slot, ob],
                                                      in_=stg)
                                else:
                                    nc.scalar.dma_start(out=out_d[slot, ob],
                                                        in_=stg)
                            elif ob % 2 == 0:
                                nc.vector.tensor_copy(stg, p_ps)
                                nc.sync.dma_start(out=out_d[slot, ob], in_=stg)
                            else:
                                nc.scalar.copy(stg, p_ps)
                                nc.scalar.dma_start(out=out_d[slot, ob],
                                                    in_=stg)
                        yield fin

                # ---- qb0: pv matmuls over the precomputed exps (no exp or
                # vector dependency), norm chains interleaved into (1,0)
                pv0 = {}

                def emit_pv0(h):
                    pv = pvps.tile([128, QBW], F32, tag="pv",
                                   name=f"pv0_{h}")
                    for sc in range(4):
                        lo = 128 * sc
                        cols = ds(lo, QBW - lo)
                        nc.tensor.matmul(pv[:, cols], v_t[:, sc, :],
                                         e0[h][sc][:, cols],
                                         start=(sc == 0), stop=(sc == 3))
                    pv0[h] = pv

                def n0_a(h):
                    l_ps = normps.tile([1, QBW], F32, tag="n",
                                       name=f"l0_{h}")
                    nc.tensor.matmul(l_ps, onesc_t, sum0[h],
                                     start=True, stop=True)
                    l_sb = lp.tile([1, QBW], BF16, tag="lsb", name="lsb")
                    nc.scalar.copy(l_sb, l_ps)
                    n0_lsb[h] = l_sb

                def n0_b(h):
                    bc_ps = normps.tile([128, QBW], F32, tag="n",
                                        name=f"bc0_{h}")
                    rc_t = rcp.tile([128, QBW], F32, tag="rc", name="rc")
                    nc.tensor.matmul(bc_ps, onesr_t, n0_lsb[h],
                                     start=True, stop=True)
                    nc.vector.reciprocal_approx_fast(out=rc_t, in_=bc_ps)
                    outT_t = outTp.tile([128, QBW], BF16, tag="outT",
                                        name=f"outT0_{h}")
                    nc.vector.tensor_mul(outT_t, pv0[h], rc_t)
                    outT_tiles[(0, h)] = outT_t

                # pv matmuls keep the PE busy while each norm chain's
                # cross-engine waits (lcopy -> bc -> recip) resolve. n0_b(2)
                # must be emitted before (1,0)'s pv tile allocation (2-buf
                # pvps WAR) to keep the in-order PE stream deadlock-free.
                n0_lsb = {}
                emit_pv0(0)
                n0_a(0)
                emit_pv0(1)
                n0_a(1)
                n0_b(0)
                emit_pv0(2)
                n0_b(1)
                emit_pv0(3)
                n0_a(2)
                n0_b(2)
                pre_units = [lambda: n0_a(3), lambda: n0_b(3)]

                def norm_pieces(qb, h, sum_t, pv_ps):
                    """Deferred norm chain for (qb,h), run inside the next
                    head's chunk stream so the in-order PE never waits on
                    the cross-engine hops at a head boundary."""
                    def p1(qb=qb, h=h, sum_t=sum_t):
                        l_ps = normps.tile([1, QBW], F32, tag="n", name="l")
                        nc.tensor.matmul(l_ps, onesc_t, sum_t,
                                         start=True, stop=True)
                        l_sb = lp.tile([1, QBW], BF16, tag="lsb",
                                       name="lsb")
                        nc.scalar.copy(l_sb, l_ps)
                        pend_l[(qb, h)] = l_sb

                    def p2(qb=qb, h=h, pv_ps=pv_ps):
                        bc_ps = normps.tile([128, QBW], F32, tag="n",
                                            name="bc")
                        nc.tensor.matmul(bc_ps, onesr_t, pend_l[(qb, h)],
                                         start=True, stop=True)
                        rc_t = rcp.tile([128, QBW], F32, tag="rc",
                                        name="rc")
                        nc.vector.reciprocal_approx_fast(out=rc_t,
                                                         in_=bc_ps)
                        outT_t = outTp.tile([128, QBW], BF16, tag="outT",
                                            name=f"outT{qb}_{h}")
                        nc.vector.tensor_mul(outT_t, pv_ps, rc_t)
                        outT_tiles[(qb, h)] = outT_t
                    return [p1, p2]

                pend_l = {}
                carried = []
                for qb in range(1, NQB):
                    for h in range(HPC):
                        nsc = 4 * (qb + 1)
                        items = carried + list(proj_items(qb - 1, h))
                        carried = []
                        if qb == 1 and h == 0:
                            items = pre_units + items
                        hold = 0
                        n_inter = len(items) - hold
                        per = -(-n_inter // nsc) if n_inter else 0
                        it = 0
                        sum_t = sump.tile([128, QBW], BF16, tag="sum",
                                          name=f"sum{qb}_{h}")
                        pv_ps = pvps.tile([128, QBW], F32, tag="pv",
                                          name=f"pv{qb}_{h}")
                        pend_pair = None
                        for sc in range(nsc):
                            t = sc - 4 * qb
                            lo = 128 * t if t >= 0 else 0
                            cols = ds(lo, QBW - lo)
                            s_ps = scps.tile([128, QBW], F32, tag="sc",
                                             name="sc")
                            nc.tensor.matmul(
                                s_ps[:, cols],
                                kt_sb[sc // 4][:, ts(sc % 4, 128)],
                                q_sb[qb][:, h, ds(lo, QBW - lo)],
                                start=True, stop=(t < 0))
                            if t >= 0:
                                # causal mask: accumulate -1e5 upper-triangle
                                # into the diagonal 128-col slice (PE-local)
                                nc.tensor.matmul(s_ps[:, ds(lo, 128)], idb_t,
                                                 hm_t, start=False, stop=True)
                            e_t = expp.tile([128, QBW], BF16, tag="exp",
                                            name="exp")
                            nc.scalar.activation(
                                e_t[:, cols], s_ps[:, cols],
                                mybir.ActivationFunctionType.Exp)
                            if t >= 0:
                                if sc == 0:
                                    nc.vector.tensor_copy(sum_t, e_t)
                                else:
                                    nc.vector.tensor_add(
                                        sum_t[:, cols], sum_t[:, cols],
                                        e_t[:, cols])
                            elif pend_pair is None:
                                pend_pair = e_t
                            else:
                                # all-bf16 pair add runs at 2x DVE rate,
                                # halving the f32 accumulation chain
                                p_t = pairp.tile([128, QBW], BF16, tag="p",
                                                 name="p")
                                nc.vector.tensor_add(p_t, pend_pair, e_t)
                                pend_pair = None
                                if sc == 1:
                                    nc.vector.tensor_copy(sum_t, p_t)
                                else:
                                    nc.vector.tensor_add(sum_t, sum_t, p_t)
                            nc.tensor.matmul(pv_ps[:, cols], v_t[:, sc, :],
                                             e_t[:, cols],
                                             start=(sc == 0),
                                             stop=(sc == nsc - 1))
                            for _ in range(per):
                                if it < n_inter:
                                    items[it]()
                                    it += 1
                        while it < len(items):
                            items[it]()
                            it += 1
                        carried = norm_pieces(qb, h, sum_t, pv_ps)
                # tail: the last q-block's projection, ob-major across the
                # 4 slots so stores spread evenly and the final drain is one
                # fine-sliced ob group instead of a whole slot
                gens = [proj_items(NQB - 1, qs, tail=True)
                        for qs in range(HPC)]
                first = [[next(gens[qs]) for _ in range(5)]
                         for qs in range(3)]
                carried[0]()                  # (3,3) l matmul + l copy
                for qs in range(3):
                    for k in range(3):        # h0-h2 matmuls of 3 groups
                        first[qs][k]()
                carried[1]()                  # (3,3) bc + recip + mul
                for qs in range(3):
                    first[qs][3]()            # h3 matmuls + fins
                    first[qs][4]()
                for _ in range(5):
                    next(gens[3])()           # group (qs=3, ob=0)
                for ob in range(1, NOB):
                    for qs in range(HPC):
                        for _ in range(5):
                            next(gens[qs])()

    nc.compile()
    return nc


def _host_prep(x, wq, wk, wv, wo, freqs_cos, freqs_sin):
    x = np.asarray(x, np.float32)
    wq = np.asarray(wq, np.float32)
    wk = np.asarray(wk, np.float32)
    wv = np.asarray(wv, np.float32)
    wo = np.asarray(wo, np.float32)
    cos = np.asarray(freqs_cos, np.float32)
    sin = np.asarray(freqs_sin, np.float32)

    scale = 1.0 / np.sqrt(np.float32(HD))
    perm = np.concatenate([np.arange(0, HD, 2), np.arange(1, HD, 2)])
    wq_p = ((wq.reshape(N_HEADS, HD, DIM)[:, perm, :])
            .reshape(DIM, DIM) * scale)
    wk_p = (wk.reshape(N_KV, HD, DIM)[:, perm, :]).reshape(N_KV * HD, DIM)

    # x tiled: xT[sb, q, p, k, s] = x[0, sb*SBW+s, (q*KQ+k)*128+p]
    xs = x.reshape(S, DIM)
    xT_tiled = np.ascontiguousarray(
        xs.reshape(NSB, SBW, NXQ, KQ, 128).transpose(0, 2, 4, 3, 1)
    ).astype(NPBF16)

    def wtile(wmat_rows):  # [128, DIM] -> [128, KCH, 128] bf16
        return np.ascontiguousarray(
            wmat_rows.T.reshape(KCH, 128, wmat_rows.shape[0])
            .transpose(1, 0, 2)).astype(NPBF16)

    cos2 = np.ascontiguousarray(np.concatenate([cos.T, cos.T], 0))
    sin2 = np.ascontiguousarray(np.concatenate([sin.T, sin.T], 0))
    sgnv = np.concatenate([-np.ones((64, 1), np.float32),
                           np.ones((64, 1), np.float32)])
    onesc_a = np.ones((128, 1), np.float32)
    onesr_a = np.ones((1, 128), np.float32)
    hm_a = np.where(np.arange(128)[:, None] > np.arange(128)[None, :],
                    np.float32(-1e5), np.float32(0.0)).astype(NPBF16)
    identb_a = np.eye(128, dtype=np.float32).astype(NPBF16)

    in_maps = []
    for c in range(NCORES):
        wq_c = wq_p[c * FEAT:(c + 1) * FEAT]
        wq_tiles = [wtile(wq_c[h * HD:(h + 1) * HD]) for h in range(HPC)]
        # wAll[half, p, obi, j, :]: obi order [k, q0, v, q1, q2, q3],
        # half h covers contraction chunks h*16 .. h*16+15
        mats = [wtile(wk_p[c * HD:(c + 1) * HD]), wq_tiles[0],
                wtile(wv[c * HD:(c + 1) * HD]), wq_tiles[1],
                wq_tiles[2], wq_tiles[3]]
        wall = np.stack(mats, axis=1)              # [128, 6, 32, 128]
        wall = wall.reshape(128, 6, 2, 16, HD).transpose(2, 0, 1, 3, 4)
        woc = wo[:, c * FEAT:(c + 1) * FEAT].T  # [FEAT, DIM]
        wo_tiled = np.ascontiguousarray(
            woc.reshape(HPC, 128, DIM)).astype(NPBF16)
        in_maps.append({
            "xT": xT_tiled,
            "wAllT": np.ascontiguousarray(wall),
            "woT": wo_tiled,
            "cos2": cos2,
            "sin2": sin2,
            "sgn": sgnv,
            "onesc": onesc_a.astype(NPBF16),
            "onesr": onesr_a.astype(NPBF16),
            "hm128": hm_a,
            "identb": identb_a,
        })
    return in_maps


def kernel(x, wq, wk, wv, wo, freqs_cos, freqs_sin, _trace=False):
    if "nc" not in _CACHE:
        _CACHE["nc"] = _build()
    nc = _CACHE["nc"]
    in_maps = _host_prep(x, wq, wk, wv, wo, freqs_cos, freqs_sin)
    res = run_bass_kernel_spmd(nc, in_maps, core_ids=list(range(NCORES)),
                               trace=_trace)
    _CACHE["last_result"] = res
    total = np.zeros((NQB * HPC, NOB, 128, OBW), np.float32)
    for c in range(NCORES):
        total += res.results[c]["out"].astype(np.float32)
    return np.ascontiguousarray(total.transpose(0, 2, 1, 3)).reshape(
        1, S, DIM)


# revision 33
# speedup vs baseline: 1.0022x; 1.0022x over previous
"""TRN2 Bass kernel for GQA attention (nn_Attention_19533511262498).

Tensor-parallel over heads across 8 NeuronCores: core c owns q-heads
[4c, 4c+4) and kv-head c (wq/wk/wv sharded on the head dim, wo on its
input dim). Each core computes a partial [S, DIM] output (bf16); the
host sums the 8 partials in f32. All matmuls run bf16 x bf16;
end-to-end max-rel error vs the f32 reference is ~5e-3 (gate 2e-2).

Phase 1 (projections + RoPE): s-blocks of 512, x streamed as k-quarter
tiles; all 6 output blocks (4 q heads, k, v) accumulate concurrently in
6 PSUM banks. All weights live in one half-major packed tensor
(chunks 0-15 / 16-31) so each 0.5MB load lands just before its first
PE deadline across the two hwdge queues; cos/sin are preloaded in full
before wo so RoPE never waits on DMA. RoPE runs entirely on the vector
engine: two half-tile muls with partition-shifted outputs read the
PSUM directly (no SBUF-SBUF swap DMAs). V is transposed by SBUF->SBUF
XBAR DMA transposes as each s-block finishes (no PE transposes).
q-block 0's attention scores+exps+bf16 col-sums are injected between
s-block boundaries of sb1/sb2 (they only need sb0 data and the spare
scps PSUM banks, and must not interleave inside an open accumulation
quarter - that corrupts the qkv PSUM groups on hardware).

Phase 2 (attention + output projection): causal triangle at 128-column
granularity: diagonal kv-chunks compute only the valid column suffix,
with the [128,128] triangle mask accumulated by a PE-local idb x hm
matmul (cheaper in practice than any cross-engine mask: the
score->exp chain stays on-engine; DVE and gpsimd variants both lost
more to chain latency than they saved in PE cycles). Softmax sums
accumulate directly in bf16 (2x DVE rate, and the column-sum matmul
needs bf16 moving anyway); normalizer = ones-column reduce matmul ->
broadcast matmul -> vector reciprocal+mul, with the l copy on the
scalar engine and the held-back half of the projection items bridging
each cross-engine wait. q-block 0 opens phase 2 with dependency-free
pv matmuls over the precomputed exps; its bc/recip/mul chains
interleave between them. The previous q-block's output projection
matmuls are interleaved INTO the score/pv chunk stream so the PE never
waits on the scalar engine's exp chain. The tail (last q-block's
projection, which needs all four heads' outT) runs ob-major across the
4 row-slots with the first three groups' h0-h2 matmuls hoisted ahead
of the last outT's norm chain, and its stores fine-sliced across both
queues. The score-PSUM pool is allocated before the phase-1 QKV pool
(banks 0-1, held across phases) so phase 2 has no WAR on phase-1
banks.

Measured ~389us HW exec (from a 407us baseline); PE active ~360us vs
~345us of pure matmul cycles, remaining idle dominated by the fixed
~7us boot, ~7us teardown, and the cold-DMA startup ramp (finer
startup DMA slicing consistently regressed: more in-flight partial
writes per tile add per-instruction semaphore overhead downstream).
"""

import ml_dtypes
import numpy as np

import concourse.bacc as bacc
import concourse.tile as tile
from concourse import mybir
from concourse.bass import ts, ds
from concourse.bass_utils import run_bass_kernel_spmd

F32 = mybir.dt.float32
BF16 = mybir.dt.bfloat16
NPBF16 = ml_dtypes.bfloat16

# problem geometry (hardcoded per contest rules)
S = 2048
DIM = 4096
HD = 128
N_HEADS = 32
N_KV = 8
NCORES = 8
HPC = N_HEADS // NCORES       # 4 q heads per core
FEAT = HPC * HD               # 512 per-core attention feature width

SBW = 512                     # phase-1 s-block width
NSB = S // SBW                # 4
KCH = DIM // 128              # 32 contraction chunks
KQ = 8                        # k-chunks per x quarter tile
NXQ = KCH // KQ               # 4
QBW = 512                     # attention q-block width
NQB = S // QBW                # 4
NSC = S // 128                # 16 kv chunks
OBW = 512                     # output-dim block width
NOB = DIM // OBW              # 8
OBS = ["k", 0, "v", 1, 2, 3]  # per-sb output-block order (k first)

_CACHE = {}


def _build():
    nc = bacc.Bacc("TRN2", target_bir_lowering=False, debug=False,
                   num_devices=NCORES)

    xT = nc.dram_tensor("xT", [NSB, NXQ, 128, KQ, SBW], BF16,
                        kind="ExternalInput").ap()
    wAllT = nc.dram_tensor("wAllT", [2, 128, 6, 16, HD], BF16,
                           kind="ExternalInput").ap()
    woT = nc.dram_tensor("woT", [HPC, 128, DIM], BF16, kind="ExternalInput").ap()
    cos2 = nc.dram_tensor("cos2", [128, S], F32, kind="ExternalInput").ap()
    sin2 = nc.dram_tensor("sin2", [128, S], F32, kind="ExternalInput").ap()
    sgn = nc.dram_tensor("sgn", [128, 1], F32, kind="ExternalInput").ap()
    onesc = nc.dram_tensor("onesc", [128, 1], BF16, kind="ExternalInput").ap()
    onesr = nc.dram_tensor("onesr", [1, 128], BF16, kind="ExternalInput").ap()
    hm128 = nc.dram_tensor("hm128", [128, 128], BF16, kind="ExternalInput").ap()
    identb = nc.dram_tensor("identb", [128, 128], BF16,
                            kind="ExternalInput").ap()
    out_d = nc.dram_tensor("out", [NQB * HPC, NOB, 128, OBW], BF16,
                           kind="ExternalOutput").ap()

    with tile.TileContext(nc) as tc:
        with (
            tc.tile_pool(name="res", bufs=1) as res,
            tc.tile_pool(name="wo", bufs=1) as wop,
            # scps first so it lands in PSUM banks 0-1 and carries across
            # both phases (phase-2/injected scores never WAR phase-1 banks)
            tc.tile_pool(name="scps", bufs=2, space="PSUM") as scps,
        ):
            kt_sb = [res.tile([128, SBW], BF16, tag=f"kt{i}", name=f"kt{i}")
                     for i in range(NSB)]
            vt_sb = [res.tile([128, SBW], BF16, tag=f"vt{i}", name=f"vt{i}")
                     for i in range(NSB)]
            q_sb = [res.tile([128, HPC, SBW], BF16, tag=f"q{i}", name=f"q{i}")
                    for i in range(NSB)]
            v_t = res.tile([128, NSC, HD], BF16, tag="v")
            sgn_t = res.tile([128, 1], F32, tag="sgn")
            onesc_t = res.tile([128, 1], BF16, tag="onesc")
            onesr_t = res.tile([1, 128], BF16, tag="onesr")
            hm_t = res.tile([128, 128], BF16, tag="hm")
            idb_t = res.tile([128, 128], BF16, tag="idb")
            cos_t = res.tile([128, S], F32, tag="cos")
            sin_t = res.tile([128, S], F32, tag="sin")
            # qb0 precompute: exps, f32 col-sums, bf16 col-sums
            e0 = [[res.tile([128, QBW], BF16, tag=f"e0_{h}_{sc}",
                            name=f"e0_{h}_{sc}") for sc in range(4)]
                  for h in range(HPC)]
            sum0 = [res.tile([128, QBW], BF16, tag=f"sum0_{h}",
                             name=f"sum0_{h}") for h in range(HPC)]

            # ---------------- Phase 1: QKV projections + RoPE ----------------
            with (
                tc.tile_pool(name="wkv", bufs=1) as wkvp,
                tc.tile_pool(name="xt", bufs=5) as xtp,
                tc.tile_pool(name="rope", bufs=3) as ropep,
                tc.tile_pool(name="qkvps", bufs=6, space="PSUM") as qkvps,
            ):
                wall_t = wkvp.tile([128, 2, 6, 16, HD], BF16, tag="wall")
                xq_tiles = {}

                def load_xq(sb, q, fine=False):
                    t = xtp.tile([128, KQ, SBW], BF16, tag="xt",
                                 name=f"x{sb}_{q}")
                    if fine:
                        for i in range(4):
                            eng = nc.scalar if i % 2 == 0 else nc.sync
                            eng.dma_start(out=t[:, 2 * i:2 * i + 2],
                                          in_=xT[sb, q, :, 2 * i:2 * i + 2])
                    else:
                        nc.sync.dma_start(out=t[:, 0:4], in_=xT[sb, q, :, 0:4])
                        nc.sync.dma_start(out=t[:, 4:8], in_=xT[sb, q, :, 4:8])
                    xq_tiles[(sb, q)] = t

                # half-major weight loads (A = chunks 0-15, B = 16-31),
                # ordered so every ob's A-half beats its first-quarter PE
                # deadline; obi order matches OBS consumption order
                def wload(h, obi, eng, part=None):
                    if part is None:
                        eng.dma_start(out=wall_t[:, h, obi],
                                      in_=wAllT[h, :, obi])
                    else:
                        j0, j1 = part
                        eng.dma_start(out=wall_t[:, h, obi, j0:j1],
                                      in_=wAllT[h, :, obi, j0:j1])

                wload(0, 0, nc.sync)          # A-k
                load_xq(0, 0, fine=True)      # x00 split scalar/sync
                wload(0, 1, nc.scalar)        # A-q0
                wload(0, 3, nc.sync)          # A-q1
                wload(0, 2, nc.scalar)        # A-v
                wload(0, 5, nc.sync)          # A-q3
                wload(0, 4, nc.scalar)        # A-q2
                load_xq(0, 1)                 # x01 on sync
                nc.scalar.dma_start(out=sgn_t, in_=sgn)
                nc.scalar.dma_start(out=onesc_t, in_=onesc)
                nc.scalar.dma_start(out=onesr_t, in_=onesr)
                nc.scalar.dma_start(out=hm_t, in_=hm128)
                nc.scalar.dma_start(out=idb_t, in_=identb)
                load_xq(0, 2)
                wload(1, 0, nc.sync)          # B-k
                wload(1, 1, nc.scalar)        # B-q0
                wload(1, 3, nc.sync)          # B-q1
                wload(1, 2, nc.scalar)        # B-v
                wload(1, 5, nc.sync)          # B-q3
                wload(1, 4, nc.scalar)        # B-q2
                load_xq(0, 3)
                # full trig preload (before wo) so RoPE never waits on DMA
                nc.scalar.dma_start(out=cos_t[:, 0:1024], in_=cos2[:, 0:1024])
                nc.scalar.dma_start(out=cos_t[:, 1024:S], in_=cos2[:, 1024:S])
                nc.scalar.dma_start(out=sin_t[:, 0:1024], in_=sin2[:, 0:1024])
                nc.scalar.dma_start(out=sin_t[:, 1024:S], in_=sin2[:, 1024:S])
                # wo prefetch (4MB bf16) behind the trig
                wo_hs = []
                for h in range(HPC):
                    wo_h = wop.tile([128, DIM], BF16, tag=f"wo{h}")
                    nc.scalar.dma_start(out=wo_h, in_=woT[h])
                    wo_hs.append(wo_h)

                seq = [(sb, q) for sb in range(NSB) for q in range(NXQ)]
                issued = 4  # all of sb0's quarters already issued

                OBI = {"k": 0, 0: 1, "v": 2, 1: 3, 2: 4, 3: 5}

                def wsrc(ob, kc):
                    return wall_t[:, kc // 16, OBI[ob], kc % 16, :]

                # qb0 score/exp/sum injections: emitted between phase-1
                # quarters once sb0's k/q are ready
                def qb0_chunk(h, sc):
                    lo = 128 * sc
                    cols = ds(lo, QBW - lo)
                    s_ps = scps.tile([128, QBW], F32, tag="sc",
                                     name=f"sc0_{h}_{sc}")
                    nc.tensor.matmul(s_ps[:, cols],
                                     kt_sb[0][:, ts(sc, 128)],
                                     q_sb[0][:, h, cols],
                                     start=True, stop=False)
                    nc.tensor.matmul(s_ps[:, ds(lo, 128)], idb_t, hm_t,
                                     start=False, stop=True)
                    nc.scalar.activation(e0[h][sc][:, cols], s_ps[:, cols],
                                         mybir.ActivationFunctionType.Exp)
                    if sc == 0:
                        nc.vector.tensor_copy(sum0[h], e0[h][0])
                    else:
                        nc.vector.tensor_add(sum0[h][:, cols],
                                             sum0[h][:, cols],
                                             e0[h][sc][:, cols])

                inj = []
                for h in range(HPC):
                    for sc in range(4):
                        inj.append(lambda h=h, sc=sc: qb0_chunk(h, sc))

                inj_per_sb = {1: 6, 2: 6}  # rest at phase-1 end
                inj_it = 0

                for sb in range(NSB):
                    c_sl = cos_t[:, ts(sb, SBW)]
                    s_sl = sin_t[:, ts(sb, SBW)]
                    ps = {ob: qkvps.tile([128, SBW], F32, tag="ps",
                                         name=f"ps{sb}_{ob}")
                          for ob in OBS}
                    for q in range(NXQ):
                        # keep 3 quarters of prefetch in flight
                        while issued < len(seq) and issued <= sb * NXQ + q + 3:
                            load_xq(*seq[issued])
                            issued += 1
                        xt = xq_tiles.pop((sb, q))
                        for ob in OBS:
                            for k in range(KQ):
                                kc = q * KQ + k
                                nc.tensor.matmul(ps[ob], wsrc(ob, kc),
                                                 xt[:, k, :],
                                                 start=(kc == 0),
                                                 stop=(kc == KCH - 1))
                    # RoPE / copies in OBS order
                    # dst = m1 + sgn*m2s, m1 = ps*cos,
                    # m2s = half-swapped ps*sin (shifted-output DVE muls)
                    for ob in OBS:
                        if ob == "v":
                            nc.vector.tensor_copy(vt_sb[sb], ps[ob])
                            # XBAR transpose v into [s, hd] chunks
                            for j in range(4):
                                eng = nc.scalar if j % 2 == 0 else nc.sync
                                eng.dma_start(
                                    out=v_t[:, 4 * sb + j, :],
                                    in_=vt_sb[sb][:, ts(j, 128)],
                                    transpose=True)
                            continue
                        m1 = ropep.tile([128, SBW], F32, tag="m1", name="m1")
                        m2s = ropep.tile([128, SBW], F32, tag="m2s",
                                         name="m2s")
                        nc.vector.tensor_mul(m1, ps[ob], c_sl)
                        nc.vector.tensor_tensor(
                            m2s[0:64], ps[ob][64:128], s_sl[64:128],
                            op=mybir.AluOpType.mult)
                        nc.vector.tensor_tensor(
                            m2s[64:128], ps[ob][0:64], s_sl[0:64],
                            op=mybir.AluOpType.mult)
                        dst = (kt_sb[sb] if ob == "k"
                               else q_sb[sb][:, ob, :])
                        nc.vector.scalar_tensor_tensor(
                            dst, m2s, sgn_t, m1,
                            op0=mybir.AluOpType.mult, op1=mybir.AluOpType.add)
                    for _ in range(inj_per_sb.get(sb, 0)):
                        if inj_it < len(inj):
                            inj[inj_it]()
                            inj_it += 1
                while inj_it < len(inj):
                    inj[inj_it]()
                    inj_it += 1

            # ---------------- Phase 2: attention + output projection --------
            with (
                tc.tile_pool(name="exp", bufs=6) as expp,
                tc.tile_pool(name="pair", bufs=3) as pairp,
                tc.tile_pool(name="sum", bufs=3) as sump,
                tc.tile_pool(name="outT", bufs=6) as outTp,
                tc.tile_pool(name="rc", bufs=2) as rcp,
                tc.tile_pool(name="lsb", bufs=2) as lp,
                tc.tile_pool(name="ost", bufs=6) as ostp,
                tc.tile_pool(name="pvps", bufs=2, space="PSUM") as pvps,
                tc.tile_pool(name="normps", bufs=1, space="PSUM") as normps,
                tc.tile_pool(name="prps", bufs=3, space="PSUM") as prps,
            ):
                outT_tiles = {}

                def proj_items(pqb, qs, tail=False):
                    """Yield emit-fns for output-projection slot (pqb, qs)."""
                    slot = pqb * HPC + qs
                    for ob in range(NOB):
                        p_ps = prps.tile([128, OBW], F32, tag="pr",
                                         name=f"pr{pqb}_{qs}_{ob}")
                        for h2 in range(HPC):
                            yield lambda ob=ob, h2=h2, p_ps=p_ps: \
                                nc.tensor.matmul(
                                    p_ps,
                                    outT_tiles[(pqb, h2)][:, ts(qs, 128)],
                                    wo_hs[h2][:, ts(ob, OBW)],
                                    start=(h2 == 0),
                                    stop=(h2 == HPC - 1))

                        # PSUM->SBUF copy (alternate vector/scalar), then a
                        # contiguous 128KB tile store (alternate sync/scalar)
                        def fin(ob=ob, p_ps=p_ps, slot=slot):
                            stg = ostp.tile([128, OBW], BF16, tag="ost",
                                            name="ost")
                            if tail:
                                # vector does all tail copies (scalar queue
                                # must stay free to issue the final stores)
                                nc.vector.tensor_copy(stg, p_ps)
                                if ob >= NOB - 2:
                                    # last groups drain through both queues
                                    nc.sync.dma_start(
                                        out=out_d[slot, ob][:, 0:256],
                                        in_=stg[:, 0:256])
                                    nc.scalar.dma_start(
                                        out=out_d[slot, ob][:, 256:512],
                                        in_=stg[:, 256:512])
                                elif ob % 2 == 0:
                                    nc.sync.dma_start(out=out_d[slot, ob],
                                                      in_=stg)
                                else:
                                    nc.scalar.dma_start(out=out_d[slot, ob],
                                                        in_=stg)
                            elif ob % 2 == 0:
                                nc.vector.tensor_copy(stg, p_ps)
                                nc.sync.dma_start(out=out_d[slot, ob], in_=stg)
                            else:
                                nc.scalar.copy(stg, p_ps)
                                nc.scalar.dma_start(out=out_d[slot, ob],
                                                    in_=stg)
                        yield fin

                # ---- qb0: pv matmuls over the precomputed exps (no exp or
                # vector dependency), norm chains interleaved into (1,0)
                pv0 = {}

                def emit_pv0(h):
                    pv = pvps.tile([128, QBW], F32, tag="pv",
                                   name=f"pv0_{h}")
                    for sc in range(4):
                        lo = 128 * sc
                        cols = ds(lo, QBW - lo)
                        nc.tensor.matmul(pv[:, cols], v_t[:, sc, :],
                                         e0[h][sc][:, cols],
                                         start=(sc == 0), stop=(sc == 3))
                    pv0[h] = pv

                def n0_a(h):
                    l_ps = normps.tile([1, QBW], F32, tag="n",
                                       name=f"l0_{h}")
                    nc.tensor.matmul(l_ps, onesc_t, sum0[h],
                                     start=True, stop=True)
                    l_sb = lp.tile([1, QBW], BF16, tag="lsb", name="lsb")
                    nc.scalar.copy(l_sb, l_ps)
                    n0_lsb[h] = l_sb

                def n0_b(h):
                    bc_ps = normps.tile([128, QBW], F32, tag="n",
                                        name=f"bc0_{h}")
                    rc_t = rcp.tile([128, QBW], F32, tag="rc", name="rc")
                    nc.tensor.matmul(bc_ps, onesr_t, n0_lsb[h],
                                     start=True, stop=True)
                    nc.vector.reciprocal_approx_fast(out=rc_t, in_=bc_ps)
                    outT_t = outTp.tile([128, QBW], BF16, tag="outT",
                                        name=f"outT0_{h}")
                    nc.vector.tensor_mul(outT_t, pv0[h], rc_t)
                    outT_tiles[(0, h)] = outT_t

                # pv matmuls keep the PE busy while each norm chain's
                # cross-engine waits (lcopy -> bc -> recip) resolve. n0_b(2)
                # must be emitted before (1,0)'s pv tile allocation (2-buf
                # pvps WAR) to keep the in-order PE stream deadlock-free.
                n0_lsb = {}
                emit_pv0(0)
                n0_a(0)
                emit_pv0(1)
                n0_a(1)
                n0_b(0)
                emit_pv0(2)
                n0_b(1)
                emit_pv0(3)
                n0_a(2)
                n0_b(2)
                pre_units = [lambda: n0_a(3), lambda: n0_b(3)]

                for qb in range(1, NQB):
                    for h in range(HPC):
                        nsc = 4 * (qb + 1)
                        items = list(proj_items(qb - 1, h))
                        if qb == 1 and h == 0:
                            items = pre_units + items
                        # hold back items to fill BOTH stalls of the
                        # normalizer chain: sum-cast -> l-matmul, and
                        # l-copy -> bc-matmul
                        hold = min(16, len(items))
                        n_inter = len(items) - hold
                        # Bresenham spread: ceil-per exhausts items at
                        # chunk 12 of qb3's 16-chunk loops, leaving the
                        # trailing chunks ACT-bound; even distribution keeps
                        # the PE ahead of the exp stream the whole loop
                        quota = [((sc + 1) * n_inter) // nsc
                                 - (sc * n_inter) // nsc
                                 for sc in range(nsc)]
                        it = 0
                        sum_t = sump.tile([128, QBW], BF16, tag="sum",
                                          name=f"sum{qb}_{h}")
                        pv_ps = pvps.tile([128, QBW], F32, tag="pv",
                                          name=f"pv{qb}_{h}")
                        pend_pair = None
                        for sc in range(nsc):
                            t = sc - 4 * qb
                            lo = 128 * t if t >= 0 else 0
                            cols = ds(lo, QBW - lo)
                            s_ps = scps.tile([128, QBW], F32, tag="sc",
                                             name="sc")
                            nc.tensor.matmul(
                                s_ps[:, cols],
                                kt_sb[sc // 4][:, ts(sc % 4, 128)],
                                q_sb[qb][:, h, ds(lo, QBW - lo)],
                                start=True, stop=(t < 0))
                            if t >= 0:
                                # causal mask: accumulate -1e5 upper-triangle
                                # into the diagonal 128-col slice (PE-local)
                                nc.tensor.matmul(s_ps[:, ds(lo, 128)], idb_t,
                                                 hm_t, start=False, stop=True)
                            e_t = expp.tile([128, QBW], BF16, tag="exp",
                                            name="exp")
                            nc.scalar.activation(
                                e_t[:, cols], s_ps[:, cols],
                                mybir.ActivationFunctionType.Exp)
                            if t >= 0:
                                if sc == 0:
                                    nc.vector.tensor_copy(sum_t, e_t)
                                else:
                                    nc.vector.tensor_add(
                                        sum_t[:, cols], sum_t[:, cols],
                                        e_t[:, cols])
                            elif pend_pair is None:
                                pend_pair = e_t
                            else:
                                # all-bf16 pair add runs at 2x DVE rate,
                                # halving the f32 accumulation chain
                                p_t = pairp.tile([128, QBW], BF16, tag="p",
                                                 name="p")
                                nc.vector.tensor_add(p_t, pend_pair, e_t)
                                pend_pair = None
                                if sc == 1:
                                    nc.vector.tensor_copy(sum_t, p_t)
                                else:
                                    nc.vector.tensor_add(sum_t, sum_t, p_t)
                            nc.tensor.matmul(pv_ps[:, cols], v_t[:, sc, :],
                                             e_t[:, cols],
                                             start=(sc == 0),
                                             stop=(sc == nsc - 1))
                            for _ in range(quota[sc]):
                                if it < n_inter:
                                    items[it]()
                                    it += 1
                        while it < n_inter:
                            items[it]()
                            it += 1
                        # normalizer: l = colsum -> broadcast -> recip -> mul.
                        # sums accumulate directly in bf16 (2x DVE rate, one
                        # less cross-engine hop); l copy runs on the scalar
                        # engine; held-back proj matmuls bridge each wait
                        half_hold = it + (len(items) - it) // 2
                        while it < half_hold:
                            items[it]()
                            it += 1
                        l_ps = normps.tile([1, QBW], F32, tag="n", name="l")
                        nc.tensor.matmul(l_ps, onesc_t, sum_t,
                                         start=True, stop=True)
                        l_sb = lp.tile([1, QBW], BF16, tag="lsb", name="lsb")
                        nc.scalar.copy(l_sb, l_ps)
                        while it < len(items):
                            items[it]()
                            it += 1
                        bc_ps = normps.tile([128, QBW], F32, tag="n",
                                            name="bc")
                        nc.tensor.matmul(bc_ps, onesr_t, l_sb,
                                         start=True, stop=True)
                        rc_t = rcp.tile([128, QBW], F32, tag="rc", name="rc")
                        nc.vector.reciprocal_approx_fast(out=rc_t, in_=bc_ps)
                        outT_t = outTp.tile([128, QBW], BF16, tag="outT",
                                            name=f"outT{qb}_{h}")
                        nc.vector.tensor_mul(outT_t, pv_ps, rc_t)
                        outT_tiles[(qb, h)] = outT_t
                # tail: the last q-block's projection, ob-major across the
                # 4 slots so stores spread evenly and the final drain is one
                # fine-sliced ob group instead of a whole slot
                gens = [proj_items(NQB - 1, qs, tail=True)
                        for qs in range(HPC)]
                first = [[next(gens[qs]) for _ in range(5)]
                         for qs in range(3)]
                for qs in range(3):
                    for k in range(3):        # h0-h2 matmuls of 3 groups
                        first[qs][k]()
                for qs in range(3):
                    first[qs][3]()            # h3 matmuls + fins
                    first[qs][4]()
                for _ in range(5):
                    next(gens[3])()           # group (qs=3, ob=0)
                for ob in range(1, NOB):
                    for qs in range(HPC):
                        for _ in range(5):
                            next(gens[qs])()

    nc.compile()
    return nc


def _host_prep(x, wq, wk, wv, wo, freqs_cos, freqs_sin):
    x = np.asarray(x, np.float32)
    wq = np.asarray(wq, np.float32)
    wk = np.asarray(wk, np.float32)
    wv = np.asarray(wv, np.float32)
    wo = np.asarray(wo, np.float32)
    cos = np.asarray(freqs_cos, np.float32)
    sin = np.asarray(freqs_sin, np.float32)

    scale = 1.0 / np.sqrt(np.float32(HD))
    perm = np.concatenate([np.arange(0, HD, 2), np.arange(1, HD, 2)])
    wq_p = ((wq.reshape(N_HEADS, HD, DIM)[:, perm, :])
            .reshape(DIM, DIM) * scale)
    wk_p = (wk.reshape(N_KV, HD, DIM)[:, perm, :]).reshape(N_KV * HD, DIM)

    # x tiled: xT[sb, q, p, k, s] = x[0, sb*SBW+s, (q*KQ+k)*128+p]
    xs = x.reshape(S, DIM)
    xT_tiled = np.ascontiguousarray(
        xs.reshape(NSB, SBW, NXQ, KQ, 128).transpose(0, 2, 4, 3, 1)
    ).astype(NPBF16)

    def wtile(wmat_rows):  # [128, DIM] -> [128, KCH, 128] bf16
        return np.ascontiguousarray(
            wmat_rows.T.reshape(KCH, 128, wmat_rows.shape[0])
            .transpose(1, 0, 2)).astype(NPBF16)

    cos2 = np.ascontiguousarray(np.concatenate([cos.T, cos.T], 0))
    sin2 = np.ascontiguousarray(np.concatenate([sin.T, sin.T], 0))
    sgnv = np.concatenate([-np.ones((64, 1), np.float32),
                           np.ones((64, 1), np.float32)])
    onesc_a = np.ones((128, 1), np.float32)
    onesr_a = np.ones((1, 128), np.float32)
    hm_a = np.where(np.arange(128)[:, None] > np.arange(128)[None, :],
                    np.float32(-1e5), np.float32(0.0)).astype(NPBF16)
    identb_a = np.eye(128, dtype=np.float32).astype(NPBF16)

    in_maps = []
    for c in range(NCORES):
        wq_c = wq_p[c * FEAT:(c + 1) * FEAT]
        wq_tiles = [wtile(wq_c[h * HD:(h + 1) * HD]) for h in range(HPC)]
        # wAll[half, p, obi, j, :]: obi order [k, q0, v, q1, q2, q3],
        # half h covers contraction chunks h*16 .. h*16+15
        mats = [wtile(wk_p[c * HD:(c + 1) * HD]), wq_tiles[0],
                wtile(wv[c * HD:(c + 1) * HD]), wq_tiles[1],
                wq_tiles[2], wq_tiles[3]]
        wall = np.stack(mats, axis=1)              # [128, 6, 32, 128]
        wall = wall.reshape(128, 6, 2, 16, HD).transpose(2, 0, 1, 3, 4)
        woc = wo[:, c * FEAT:(c + 1) * FEAT].T  # [FEAT, DIM]
        wo_tiled = np.ascontiguousarray(
            woc.reshape(HPC, 128, DIM)).astype(NPBF16)
        in_maps.append({
            "xT": xT_tiled,
            "wAllT": np.ascontiguousarray(wall),
            "woT": wo_tiled,
            "cos2": cos2,
            "sin2": sin2,
            "sgn": sgnv,
            "onesc": onesc_a.astype(NPBF16),
            "onesr": onesr_a.astype(NPBF16),
            "hm128": hm_a,
            "identb": identb_a,
        })
    return in_maps


def kernel(x, wq, wk, wv, wo, freqs_cos, freqs_sin, _trace=False):
    if "nc" not in _CACHE:
        _CACHE["nc"] = _build()
    nc = _CACHE["nc"]
    in_maps = _host_prep(x, wq, wk, wv, wo, freqs_cos, freqs_sin)
    res = run_bass_kernel_spmd(nc, in_maps, core_ids=list(range(NCORES)),
                               trace=_trace)
    _CACHE["last_result"] = res
    total = np.zeros((NQB * HPC, NOB, 128, OBW), np.float32)
    for c in range(NCORES):
        total += res.results[c]["out"].astype(np.float32)
    return np.ascontiguousarray(total.transpose(0, 2, 1, 3)).reshape(
        1, S, DIM)
